# revision 18
# baseline (speedup 1.0000x reference)
"""SE (squeeze-excite) block for x[32,64,256,256] f32 on 8 TRN2 NeuronCores.

Data-parallel over batch: 4 batches per core, SE weights replicated.
Per core: x viewed as [256 rows = (4b x 64c), 65536 spatial], split into
32 chunks of [128 partitions, 4096] (2 MiB DMAs).

  Pass 1: stream chunks, DVE reduce_sum -> row sums. The last N_CACHE
          chunks stay resident in SBUF and are not re-read in pass 2.
  MLP:    two tiny PE matmuls (64->4 relu, 4->64 sigmoid); per-partition
          biases on the scalar engine; the 1/65536 mean scale is folded
          into the relu activation's scale argument.
  Pass 2: cached chunks are scaled in place and stored; the rest are
          re-streamed, scaled, stored.

HBM traffic per core: (2R + 1W - cached) * 64 MiB -> memory-bound.
"""

import numpy as np

import concourse.bacc as bacc
import concourse.bass as bass
import concourse.mybir as mybir
from concourse import tile
from concourse.bass_utils import run_bass_kernel_spmd

N_CORES = 8
B, C, H, W = 32, 64, 256, 256
C_MID = 4
B_LOC = B // N_CORES            # 4 batches per core
ROWS = B_LOC * C                # 256 (b,c) rows per core
SPATIAL = H * W                 # 65536
NG = ROWS // 128                # 2 partition groups
NB_PER_G = 128 // C             # 2 batches per partition group
T = 2048                        # spatial chunk (8KB/partition, 1MiB/DMA)
NS = SPATIAL // T               # 16 chunks per group
N_CHUNKS = NG * NS              # 32 chunks total
N_CACHE = 19                    # chunks kept resident in SBUF
N_STREAM_BUFS = 5
F32 = mybir.dt.float32

TRACE = False
LAST_RESULT = None

_NC = None


def _chunk_order():
    """(g, s) pairs in pass-1 emission order: streamed first, cached last."""
    order = [(g, s) for g in range(NG) for s in range(NS)]
    return order[:N_CHUNKS - N_CACHE], order[N_CHUNKS - N_CACHE:]


def _build():
    global _NC
    if _NC is not None:
        return _NC

    nc = bacc.Bacc("TRN2", debug=False)

    x = nc.dram_tensor("x", [ROWS, SPATIAL], F32, kind="ExternalInput")
    wd = nc.dram_tensor("w_down", [C_MID, C], F32, kind="ExternalInput")
    bd = nc.dram_tensor("b_down", [C_MID], F32, kind="ExternalInput")
    wu = nc.dram_tensor("w_up", [C, C_MID], F32, kind="ExternalInput")
    bu = nc.dram_tensor("b_up", [C], F32, kind="ExternalInput")
    y = nc.dram_tensor("y", [ROWS, SPATIAL], F32, kind="ExternalOutput")

    x_t = x.ap().rearrange("(g p) (s t) -> g p s t", p=128, t=T)
    y_t = y.ap().rearrange("(g p) (s t) -> g p s t", p=128, t=T)

    streamed, cached = _chunk_order()

    with tile.TileContext(nc) as tc:
        with (
            tc.tile_pool(name="const", bufs=1) as cpool,
            tc.tile_pool(name="io", bufs=N_STREAM_BUFS) as io_pool,
            tc.tile_pool(name="cache", bufs=N_CACHE) as cache_pool,
            tc.tile_pool(name="stats", bufs=1) as spool,
            tc.tile_pool(name="psum", bufs=1, space=bass.MemorySpace.PSUM) as ppool,
        ):
            # --- packed constants: one SBUF page ---
            # SBUF row layout is p = c + 64*h (h = batch parity in group), so
            # w_down^T and b_up are duplicated into both partition halves;
            # the PE then contracts each half separately and the sigmoid
            # output lands directly in row layout -- no transpose DMAs.
            # cols 0:4   partitions 0:128 -> w_down^T dup  [(h c), m]
            # cols 4:68  partitions 0:4   -> w_up^T        [m, c]
            # col  68    partitions 0:4   -> b_down        [m, 1]
            # col  69    partitions 0:128 -> b_up dup      [(h c), 1]
            const_t = cpool.tile([128, 70], F32)
            wdT = const_t[:, 0:C_MID]
            wuT = const_t[0:C_MID, C_MID:C_MID + C]
            bdT = const_t[0:C_MID, 68:69]
            buT = const_t[:, 69:70]
            for h in range(NB_PER_G):
                nc.sync.dma_start(wdT[h * C:(h + 1) * C, :],
                                  wd.ap().rearrange("m c -> c m"))
                nc.sync.dma_start(buT[h * C:(h + 1) * C, :], bu.ap().unsqueeze(1))
            nc.sync.dma_start(wuT, wu.ap().rearrange("c m -> m c"))
            nc.sync.dma_start(bdT, bd.ap().unsqueeze(1))

            # --- packed stats: one SBUF page (engine-written only) ---
            # cols 0:N_CHUNKS      -> per-chunk row sums [128, (g s)]
            # cols N_CHUNKS+0:+2   -> tot  [p, g] full row sums
            # cols N_CHUNKS+2:+6   (partitions 0:4) -> hT [m, (h g)]
            # cols N_CHUNKS+6:+8   -> scl [p, g] sigmoid scale per row
            stats_t = spool.tile([128, N_CHUNKS + 8], F32)
            sums = stats_t[:, 0:N_CHUNKS].rearrange("p (g s) -> p g s", g=NG)
            tot = stats_t[:, N_CHUNKS:N_CHUNKS + 2]
            hT = stats_t[0:C_MID, N_CHUNKS + 2:N_CHUNKS + 6]
            scl = stats_t[:, N_CHUNKS + 6:N_CHUNKS + 8]

            cache_tiles = {}

            # --- pass 1: row sums over spatial ---
            for g, s in streamed:
                tin = io_pool.tile([128, T], F32, tag="io")
                nc.gpsimd.dma_start(tin[:], x_t[g, :, s, :])
                nc.vector.reduce_sum(sums[:, g, s:s + 1], tin[:],
                                     axis=mybir.AxisListType.X)
            for g, s in cached:
                ct = cache_pool.tile([128, T], F32, tag="cache")
                cache_tiles[(g, s)] = ct
                nc.gpsimd.dma_start(ct[:], x_t[g, :, s, :])
                nc.vector.reduce_sum(sums[:, g, s:s + 1], ct[:],
                                     axis=mybir.AxisListType.X)
            nc.vector.reduce_sum(tot[:], sums[:], axis=mybir.AxisListType.X)

            # --- excite MLP, entirely in row layout p = c + 64h ---
            # hT[m, 2h+g] = relu(sum_c w_down[m,c] tot[64h+c, g] / 65536 + b_down[m])
            ph = ppool.tile([C_MID, NB_PER_G * NG], F32)
            for h in range(NB_PER_G):
                nc.tensor.matmul(ph[:, NG * h:NG * (h + 1)],
                                 wdT[h * C:(h + 1) * C, :],
                                 tot[h * C:(h + 1) * C, :])
            nc.scalar.activation(hT, ph[:], mybir.ActivationFunctionType.Relu,
                                 bias=bdT, scale=1.0 / float(SPATIAL))
            # ps[64h+c, g] = sum_m w_up[c,m] hT[m, 2h+g]; sigmoid -> scl
            ps = ppool.tile([128, NG], F32)
            for h in range(NB_PER_G):
                nc.tensor.matmul(ps[h * C:(h + 1) * C, :],
                                 wuT, hT[:, NG * h:NG * (h + 1)])
            nc.scalar.activation(scl, ps[:], mybir.ActivationFunctionType.Sigmoid,
                                 bias=buT, scale=1.0)

            # --- pass 2: y = x * scale[row] ---
            # cached chunks first: ready as soon as scl is, no load needed
            for g, s in cached:
                ct = cache_tiles[(g, s)]
                nc.vector.tensor_scalar_mul(ct[:], ct[:], scl[:, g:g + 1])
                nc.sync.dma_start(y_t[g, :, s, :], ct[:])
            for g, s in streamed:
                tin = io_pool.tile([128, T], F32, tag="io")
                nc.gpsimd.dma_start(tin[:], x_t[g, :, s, :])
                nc.vector.tensor_scalar_mul(tin[:], tin[:], scl[:, g:g + 1])
                nc.sync.dma_start(y_t[g, :, s, :], tin[:])

    nc.compile()
    _NC = nc
    return nc


def kernel(trans_b, w_down, b_down, w_up, b_up):
    global LAST_RESULT
    nc = _build()

    trans_b = np.ascontiguousarray(np.asarray(trans_b, dtype=np.float32))
    w_down = np.ascontiguousarray(np.asarray(w_down, dtype=np.float32))
    b_down = np.ascontiguousarray(np.asarray(b_down, dtype=np.float32))
    w_up = np.ascontiguousarray(np.asarray(w_up, dtype=np.float32))
    b_up = np.ascontiguousarray(np.asarray(b_up, dtype=np.float32))

    x_flat = trans_b.reshape(B * C, SPATIAL)
    in_maps = []
    for i in range(N_CORES):
        in_maps.append({
            "x": x_flat[i * ROWS:(i + 1) * ROWS],
            "w_down": w_down,
            "b_down": b_down,
            "w_up": w_up,
            "b_up": b_up,
        })

    res = run_bass_kernel_spmd(nc, in_maps, core_ids=list(range(N_CORES)),
                               trace=TRACE)
    LAST_RESULT = res

    out = np.concatenate([res.results[i]["y"] for i in range(N_CORES)], axis=0)
    return out.reshape(B, C, H, W)


# revision 19
# speedup vs baseline: 1.0133x; 1.0133x over previous
"""SE (squeeze-excite) block for x[32,64,256,256] f32 on 8 TRN2 NeuronCores.

Data-parallel over batch: 4 batches per core, SE weights replicated.
Per core: x viewed as [256 rows = (4b x 64c), 65536 spatial], split into
32 chunks of [128 partitions, 4096] (2 MiB DMAs).

  Pass 1: stream chunks, DVE reduce_sum -> row sums. The last N_CACHE
          chunks stay resident in SBUF and are not re-read in pass 2.
  MLP:    two tiny PE matmuls (64->4 relu, 4->64 sigmoid); per-partition
          biases on the scalar engine; the 1/65536 mean scale is folded
          into the relu activation's scale argument.
  Pass 2: cached chunks are scaled in place and stored; the rest are
          re-streamed, scaled, stored.

HBM traffic per core: (2R + 1W - cached) * 64 MiB -> memory-bound.
"""

import numpy as np

import concourse.bacc as bacc
import concourse.bass as bass
import concourse.mybir as mybir
from concourse import tile
from concourse.bass_utils import run_bass_kernel_spmd

N_CORES = 8
B, C, H, W = 32, 64, 256, 256
C_MID = 4
B_LOC = B // N_CORES            # 4 batches per core
ROWS = B_LOC * C                # 256 (b,c) rows per core
SPATIAL = H * W                 # 65536
NG = ROWS // 128                # 2 partition groups
NB_PER_G = 128 // C             # 2 batches per partition group
T = 2048                        # spatial chunk (8KB/partition, 1MiB/DMA)
NS = SPATIAL // T               # 16 chunks per group
N_CHUNKS = NG * NS              # 32 chunks total
N_CACHE = 19                    # chunks kept resident in SBUF
N_STREAM_BUFS = 5
F32 = mybir.dt.float32

TRACE = False
LAST_RESULT = None

_NC = None


def _chunk_order():
    """(g, s) pairs in pass-1 emission order: streamed first, cached last."""
    order = [(g, s) for g in range(NG) for s in range(NS)]
    return order[:N_CHUNKS - N_CACHE], order[N_CHUNKS - N_CACHE:]


def _build():
    global _NC
    if _NC is not None:
        return _NC

    nc = bacc.Bacc("TRN2", debug=False)

    x = nc.dram_tensor("x", [ROWS, SPATIAL], F32, kind="ExternalInput")
    wd = nc.dram_tensor("w_down", [C_MID, C], F32, kind="ExternalInput")
    bd = nc.dram_tensor("b_down", [C_MID], F32, kind="ExternalInput")
    wu = nc.dram_tensor("w_up", [C, C_MID], F32, kind="ExternalInput")
    bu = nc.dram_tensor("b_up", [C], F32, kind="ExternalInput")
    y = nc.dram_tensor("y", [ROWS, SPATIAL], F32, kind="ExternalOutput")

    x_t = x.ap().rearrange("(g p) (s t) -> g p s t", p=128, t=T)
    y_t = y.ap().rearrange("(g p) (s t) -> g p s t", p=128, t=T)

    streamed, cached = _chunk_order()

    with tile.TileContext(nc) as tc:
        with (
            tc.tile_pool(name="const", bufs=1) as cpool,
            tc.tile_pool(name="io", bufs=N_STREAM_BUFS) as io_pool,
            tc.tile_pool(name="cache", bufs=N_CACHE) as cache_pool,
            tc.tile_pool(name="stats", bufs=1) as spool,
            tc.tile_pool(name="psum", bufs=1, space=bass.MemorySpace.PSUM) as ppool,
        ):
            # --- first loads on the HW ring, ahead of everything ---
            # data starts flowing during the ~2us SWDGE warm-up
            head_tiles = []
            for g, s in streamed[:3]:
                tin = io_pool.tile([128, T], F32, tag="io")
                nc.sync.dma_start(tin[:], x_t[g, :, s, :])
                head_tiles.append(((g, s), tin))

            # --- packed constants: one SBUF page ---
            # SBUF row layout is p = c + 64*h (h = batch parity in group), so
            # w_down^T and b_up are duplicated into both partition halves;
            # the PE then contracts each half separately and the sigmoid
            # output lands directly in row layout -- no transpose DMAs.
            # cols 0:4   partitions 0:128 -> w_down^T dup  [(h c), m]
            # cols 4:68  partitions 0:4   -> w_up^T        [m, c]
            # col  68    partitions 0:4   -> b_down        [m, 1]
            # col  69    partitions 0:128 -> b_up dup      [(h c), 1]
            const_t = cpool.tile([128, 70], F32)
            wdT = const_t[:, 0:C_MID]
            wuT = const_t[0:C_MID, C_MID:C_MID + C]
            bdT = const_t[0:C_MID, 68:69]
            buT = const_t[:, 69:70]
            for h in range(NB_PER_G):
                nc.sync.dma_start(wdT[h * C:(h + 1) * C, :],
                                  wd.ap().rearrange("m c -> c m"))
                nc.sync.dma_start(buT[h * C:(h + 1) * C, :], bu.ap().unsqueeze(1))
            nc.sync.dma_start(wuT, wu.ap().rearrange("c m -> m c"))
            nc.sync.dma_start(bdT, bd.ap().unsqueeze(1))

            # --- packed stats: one SBUF page (engine-written only) ---
            # cols 0:N_CHUNKS      -> per-chunk row sums [128, (g s)]
            # cols N_CHUNKS+0:+2   -> tot  [p, g] full row sums
            # cols N_CHUNKS+2:+6   (partitions 0:4) -> hT [m, (h g)]
            # cols N_CHUNKS+6:+8   -> scl [p, g] sigmoid scale per row
            stats_t = spool.tile([128, N_CHUNKS + 8], F32)
            sums = stats_t[:, 0:N_CHUNKS].rearrange("p (g s) -> p g s", g=NG)
            tot = stats_t[:, N_CHUNKS:N_CHUNKS + 2]
            hT = stats_t[0:C_MID, N_CHUNKS + 2:N_CHUNKS + 6]
            scl = stats_t[:, N_CHUNKS + 6:N_CHUNKS + 8]

            cache_tiles = {}

            # --- pass 1: row sums over spatial ---
            for (g, s), tin in head_tiles:
                nc.vector.reduce_sum(sums[:, g, s:s + 1], tin[:],
                                     axis=mybir.AxisListType.X)
            for g, s in streamed[3:]:
                tin = io_pool.tile([128, T], F32, tag="io")
                nc.gpsimd.dma_start(tin[:], x_t[g, :, s, :])
                nc.vector.reduce_sum(sums[:, g, s:s + 1], tin[:],
                                     axis=mybir.AxisListType.X)
            for g, s in cached:
                ct = cache_pool.tile([128, T], F32, tag="cache")
                cache_tiles[(g, s)] = ct
                nc.gpsimd.dma_start(ct[:], x_t[g, :, s, :])
                nc.vector.reduce_sum(sums[:, g, s:s + 1], ct[:],
                                     axis=mybir.AxisListType.X)
            nc.vector.reduce_sum(tot[:], sums[:], axis=mybir.AxisListType.X)

            # --- excite MLP, entirely in row layout p = c + 64h ---
            # hT[m, 2h+g] = relu(sum_c w_down[m,c] tot[64h+c, g] / 65536 + b_down[m])
            ph = ppool.tile([C_MID, NB_PER_G * NG], F32)
            for h in range(NB_PER_G):
                nc.tensor.matmul(ph[:, NG * h:NG * (h + 1)],
                                 wdT[h * C:(h + 1) * C, :],
                                 tot[h * C:(h + 1) * C, :])
            nc.scalar.activation(hT, ph[:], mybir.ActivationFunctionType.Relu,
                                 bias=bdT, scale=1.0 / float(SPATIAL))
            # ps[64h+c, g] = sum_m w_up[c,m] hT[m, 2h+g]; sigmoid -> scl
            ps = ppool.tile([128, NG], F32)
            for h in range(NB_PER_G):
                nc.tensor.matmul(ps[h * C:(h + 1) * C, :],
                                 wuT, hT[:, NG * h:NG * (h + 1)])
            nc.scalar.activation(scl, ps[:], mybir.ActivationFunctionType.Sigmoid,
                                 bias=buT, scale=1.0)

            # --- pass 2: y = x * scale[row] ---
            # cached chunks first: ready as soon as scl is, no load needed
            store_engines = [nc.sync, nc.scalar]
            n_st = 0
            for g, s in cached:
                ct = cache_tiles[(g, s)]
                nc.vector.tensor_scalar_mul(ct[:], ct[:], scl[:, g:g + 1])
                store_engines[n_st % 2].dma_start(y_t[g, :, s, :], ct[:])
                n_st += 1
            for g, s in streamed:
                tin = io_pool.tile([128, T], F32, tag="io")
                nc.gpsimd.dma_start(tin[:], x_t[g, :, s, :])
                nc.vector.tensor_scalar_mul(tin[:], tin[:], scl[:, g:g + 1])
                store_engines[n_st % 2].dma_start(y_t[g, :, s, :], tin[:])
                n_st += 1

    nc.compile()
    _NC = nc
    return nc


def kernel(trans_b, w_down, b_down, w_up, b_up):
    global LAST_RESULT
    nc = _build()

    trans_b = np.ascontiguousarray(np.asarray(trans_b, dtype=np.float32))
    w_down = np.ascontiguousarray(np.asarray(w_down, dtype=np.float32))
    b_down = np.ascontiguousarray(np.asarray(b_down, dtype=np.float32))
    w_up = np.ascontiguousarray(np.asarray(w_up, dtype=np.float32))
    b_up = np.ascontiguousarray(np.asarray(b_up, dtype=np.float32))

    x_flat = trans_b.reshape(B * C, SPATIAL)
    in_maps = []
    for i in range(N_CORES):
        in_maps.append({
            "x": x_flat[i * ROWS:(i + 1) * ROWS],
            "w_down": w_down,
            "b_down": b_down,
            "w_up": w_up,
            "b_up": b_up,
        })

    res = run_bass_kernel_spmd(nc, in_maps, core_ids=list(range(N_CORES)),
                               trace=TRACE)
    LAST_RESULT = res

    out = np.concatenate([res.results[i]["y"] for i in range(N_CORES)], axis=0)
    return out.reshape(B, C, H, W)


# revision 20
# speedup vs baseline: 1.0452x; 1.0314x over previous
"""SE (squeeze-excite) block for x[32,64,256,256] f32 on 8 TRN2 NeuronCores.

Data-parallel over batch: 4 batches per core, SE weights replicated.
Per core: x viewed as [256 rows = (4b x 64c), 65536 spatial], split into
32 chunks of [128 partitions, 4096] (2 MiB DMAs).

  Pass 1: stream chunks, DVE reduce_sum -> row sums. The last N_CACHE
          chunks stay resident in SBUF and are not re-read in pass 2.
  MLP:    two tiny PE matmuls (64->4 relu, 4->64 sigmoid); per-partition
          biases on the scalar engine; the 1/65536 mean scale is folded
          into the relu activation's scale argument.
  Pass 2: cached chunks are scaled in place and stored; the rest are
          re-streamed, scaled, stored.

HBM traffic per core: (2R + 1W - cached) * 64 MiB -> memory-bound.
"""

import numpy as np

import concourse.bacc as bacc
import concourse.bass as bass
import concourse.mybir as mybir
from concourse import tile
from concourse.bass_utils import run_bass_kernel_spmd

N_CORES = 8
B, C, H, W = 32, 64, 256, 256
C_MID = 4
B_LOC = B // N_CORES            # 4 batches per core
ROWS = B_LOC * C                # 256 (b,c) rows per core
SPATIAL = H * W                 # 65536
NG = ROWS // 128                # 2 partition groups
NB_PER_G = 128 // C             # 2 batches per partition group
T = 2048                        # spatial chunk (8KB/partition, 1MiB/DMA)
NS = SPATIAL // T               # 16 chunks per group
N_CHUNKS = NG * NS              # 32 chunks total
N_CACHE = 20                    # chunks kept resident in SBUF
N_STREAM_BUFS = 4
F32 = mybir.dt.float32

TRACE = False
LAST_RESULT = None

_NC = None


def _chunk_order():
    """(g, s) pairs in pass-1 emission order: streamed first, cached last."""
    order = [(g, s) for g in range(NG) for s in range(NS)]
    return order[:N_CHUNKS - N_CACHE], order[N_CHUNKS - N_CACHE:]


def _build():
    global _NC
    if _NC is not None:
        return _NC

    nc = bacc.Bacc("TRN2", debug=False)

    x = nc.dram_tensor("x", [ROWS, SPATIAL], F32, kind="ExternalInput")
    wd = nc.dram_tensor("w_down", [C_MID, C], F32, kind="ExternalInput")
    bd = nc.dram_tensor("b_down", [C_MID], F32, kind="ExternalInput")
    wu = nc.dram_tensor("w_up", [C, C_MID], F32, kind="ExternalInput")
    bu = nc.dram_tensor("b_up", [C], F32, kind="ExternalInput")
    y = nc.dram_tensor("y", [ROWS, SPATIAL], F32, kind="ExternalOutput")

    x_t = x.ap().rearrange("(g p) (s t) -> g p s t", p=128, t=T)
    y_t = y.ap().rearrange("(g p) (s t) -> g p s t", p=128, t=T)

    streamed, cached = _chunk_order()

    with tile.TileContext(nc) as tc:
        with (
            tc.tile_pool(name="const", bufs=1) as cpool,
            tc.tile_pool(name="io", bufs=N_CACHE + N_STREAM_BUFS) as io_pool,
            tc.tile_pool(name="stats", bufs=1) as spool,
            tc.tile_pool(name="psum", bufs=1, space=bass.MemorySpace.PSUM) as ppool,
        ):
            # --- first loads on the HW ring, ahead of everything ---
            # data starts flowing during the ~2us SWDGE warm-up
            head_tiles = []
            for g, s in streamed[:3]:
                tin = io_pool.tile([128, T], F32, tag="io")
                nc.sync.dma_start(tin[:], x_t[g, :, s, :])
                head_tiles.append(((g, s), tin))

            # --- packed constants: one SBUF page ---
            # SBUF row layout is p = c + 64*h (h = batch parity in group), so
            # w_down^T and b_up are duplicated into both partition halves;
            # the PE then contracts each half separately and the sigmoid
            # output lands directly in row layout -- no transpose DMAs.
            # cols 0:4   partitions 0:128 -> w_down^T dup  [(h c), m]
            # cols 4:68  partitions 0:4   -> w_up^T        [m, c]
            # col  68    partitions 0:4   -> b_down        [m, 1]
            # col  69    partitions 0:128 -> b_up dup      [(h c), 1]
            const_t = cpool.tile([128, 70], F32)
            wdT = const_t[:, 0:C_MID]
            wuT = const_t[0:C_MID, C_MID:C_MID + C]
            bdT = const_t[0:C_MID, 68:69]
            buT = const_t[:, 69:70]
            for h in range(NB_PER_G):
                nc.sync.dma_start(wdT[h * C:(h + 1) * C, :],
                                  wd.ap().rearrange("m c -> c m"))
                nc.sync.dma_start(buT[h * C:(h + 1) * C, :], bu.ap().unsqueeze(1))
            nc.sync.dma_start(wuT, wu.ap().rearrange("c m -> m c"))
            nc.sync.dma_start(bdT, bd.ap().unsqueeze(1))

            # --- packed stats: one SBUF page (engine-written only) ---
            # cols 0:N_CHUNKS      -> per-chunk row sums [128, (g s)]
            # cols N_CHUNKS+0:+2   -> tot  [p, g] full row sums
            # cols N_CHUNKS+2:+6   (partitions 0:4) -> hT [m, (h g)]
            # cols N_CHUNKS+6:+8   -> scl [p, g] sigmoid scale per row
            stats_t = spool.tile([128, N_CHUNKS + 8], F32)
            sums = stats_t[:, 0:N_CHUNKS].rearrange("p (g s) -> p g s", g=NG)
            tot = stats_t[:, N_CHUNKS:N_CHUNKS + 2]
            hT = stats_t[0:C_MID, N_CHUNKS + 2:N_CHUNKS + 6]
            scl = stats_t[:, N_CHUNKS + 6:N_CHUNKS + 8]

            cache_tiles = {}

            # --- pass 1: row sums over spatial ---
            for (g, s), tin in head_tiles:
                nc.vector.reduce_sum(sums[:, g, s:s + 1], tin[:],
                                     axis=mybir.AxisListType.X)
            for g, s in streamed[3:]:
                tin = io_pool.tile([128, T], F32, tag="io")
                nc.gpsimd.dma_start(tin[:], x_t[g, :, s, :])
                nc.vector.reduce_sum(sums[:, g, s:s + 1], tin[:],
                                     axis=mybir.AxisListType.X)
            for g, s in cached:
                ct = io_pool.tile([128, T], F32, tag="io")
                cache_tiles[(g, s)] = ct
                nc.gpsimd.dma_start(ct[:], x_t[g, :, s, :])
                nc.vector.reduce_sum(sums[:, g, s:s + 1], ct[:],
                                     axis=mybir.AxisListType.X)
            nc.vector.reduce_sum(tot[:], sums[:], axis=mybir.AxisListType.X)

            # --- excite MLP, entirely in row layout p = c + 64h ---
            # hT[m, 2h+g] = relu(sum_c w_down[m,c] tot[64h+c, g] / 65536 + b_down[m])
            ph = ppool.tile([C_MID, NB_PER_G * NG], F32)
            for h in range(NB_PER_G):
                nc.tensor.matmul(ph[:, NG * h:NG * (h + 1)],
                                 wdT[h * C:(h + 1) * C, :],
                                 tot[h * C:(h + 1) * C, :])
            nc.scalar.activation(hT, ph[:], mybir.ActivationFunctionType.Relu,
                                 bias=bdT, scale=1.0 / float(SPATIAL))
            # ps[64h+c, g] = sum_m w_up[c,m] hT[m, 2h+g]; sigmoid -> scl
            ps = ppool.tile([128, NG], F32)
            for h in range(NB_PER_G):
                nc.tensor.matmul(ps[h * C:(h + 1) * C, :],
                                 wuT, hT[:, NG * h:NG * (h + 1)])
            nc.scalar.activation(scl, ps[:], mybir.ActivationFunctionType.Sigmoid,
                                 bias=buT, scale=1.0)

            # --- pass 2: y = x * scale[row] ---
            # cached chunks first: ready as soon as scl is, no load needed
            store_engines = [nc.sync, nc.scalar]
            n_st = 0
            for g, s in cached:
                ct = cache_tiles[(g, s)]
                nc.vector.tensor_scalar_mul(ct[:], ct[:], scl[:, g:g + 1])
                store_engines[n_st % 2].dma_start(y_t[g, :, s, :], ct[:])
                n_st += 1
            for g, s in streamed:
                tin = io_pool.tile([128, T], F32, tag="io")
                nc.gpsimd.dma_start(tin[:], x_t[g, :, s, :])
                nc.vector.tensor_scalar_mul(tin[:], tin[:], scl[:, g:g + 1])
                store_engines[n_st % 2].dma_start(y_t[g, :, s, :], tin[:])
                n_st += 1

    nc.compile()
    _NC = nc
    return nc


def kernel(trans_b, w_down, b_down, w_up, b_up):
    global LAST_RESULT
    nc = _build()

    trans_b = np.ascontiguousarray(np.asarray(trans_b, dtype=np.float32))
    w_down = np.ascontiguousarray(np.asarray(w_down, dtype=np.float32))
    b_down = np.ascontiguousarray(np.asarray(b_down, dtype=np.float32))
    w_up = np.ascontiguousarray(np.asarray(w_up, dtype=np.float32))
    b_up = np.ascontiguousarray(np.asarray(b_up, dtype=np.float32))

    x_flat = trans_b.reshape(B * C, SPATIAL)
    in_maps = []
    for i in range(N_CORES):
        in_maps.append({
            "x": x_flat[i * ROWS:(i + 1) * ROWS],
            "w_down": w_down,
            "b_down": b_down,
            "w_up": w_up,
            "b_up": b_up,
        })

    res = run_bass_kernel_spmd(nc, in_maps, core_ids=list(range(N_CORES)),
                               trace=TRACE)
    LAST_RESULT = res

    out = np.concatenate([res.results[i]["y"] for i in range(N_CORES)], axis=0)
    return out.reshape(B, C, H, W)
